# revision 19
# baseline (speedup 1.0000x reference)
"""Trainium2 Bass kernel for EquivariantGraphConvCheap (gnn_message_passing), v6.

v5 + grouped gathers:
  - Slots are grouped (GROUPS) and each group's lo/hi gathers are emitted as
    ONE dma_gather instruction each (~20 gathers/core instead of 98), cutting
    the SWDGE 994ns-per-instruction fixed cost on the GpSimd/Pool engine from
    ~175us busy to ~45us.  The same grouping batches the xt loads and out
    stores into one HWDGE DMA per group.
  - Edge indices sorted ascending within each (slot, half) segment.
  - out / xt HBM tensors are [128, nslot, D] so group slices are contiguous.
Everything else as v5 (e3m4 gather payload, flex lo/hi split, fp16 one-hot S,
flipped segment-sum matmuls, fp16 out, DVE bias add).
"""
import os
import numpy as np
import ml_dtypes

import concourse.bacc as bacc
import concourse.mybir as mybir
import concourse.tile as tile
from concourse import bass_utils

# ---- hardcoded problem geometry ----
N = 50000
E = 500000
H = 128
D = 4 * H
NCORES = 8
NSLOT = 49
LO_MAX = 32768
HI_BASE = 25000
GROUPS = tuple(int(x) for x in
               os.environ.get("KGROUPS", "1,2,3,4,6,6,7,7,7,6").split(","))
# Max chunks (128-idx units) per dma_gather instruction.  The SWDGE
# descriptor-ring carveout holds only ~128 descs per ring; one gather needs
# num_idxs/16+1 descs, so ≤13 chunks (105 descs) is the proven-safe size.
MAX_GCHUNKS = int(os.environ.get("KMAXG", "13"))
# single_packet=True concatenates an instruction's descriptors into ONE DMA
# packet; the HW packet ceiling is 64 descriptors (= 1008 idxs).  Bigger
# gathers must use one-packet-per-descriptor mode.
SINGLE_PACKET = bool(int(os.environ.get("KSP", "0")))
# 0 = one tile + one DMA per group; 1 = one tile per group, per-slot DMAs;
# 2 = per-slot tiles + per-slot DMAs (v5-equivalent)
XT_MODE = int(os.environ.get("KXT", "0"))
OSB_MODE = int(os.environ.get("KOSB", "0"))
UNBATCH_XT = XT_MODE == 1
UNBATCH_OUT = OSB_MODE == 1
assert sum(GROUPS) == NSLOT
HEAD_GROUPS = 2

f16 = mybir.dt.float16
f32 = mybir.dt.float32
f8 = mybir.dt.float8e3
i16 = mybir.dt.int16
np_f8 = ml_dtypes.float8_e3m4


def _hoist_extra_waits(nc, max_waits=1):
    n_fixed = 0
    for fn in nc.m.functions:
        for blk in fn.blocks:
            new_insts = []
            for ins in blk.instructions:
                si = ins.sync_info
                if si is not None and si.on_wait and len(si.on_wait) > max_waits:
                    waits = list(si.on_wait)
                    for j, w in enumerate(waits[:-max_waits]):
                        nop = mybir.InstNoOp(
                            name=f"{ins.name}-waitnop{j}", ins=[], outs=[])
                        nop.engine = ins.engine
                        nop.sync_info = mybir.SyncInfo(on_wait=[w], on_update=[])
                        new_insts.append(nop)
                    ins.sync_info = mybir.SyncInfo(
                        on_wait=waits[-max_waits:],
                        on_update=list(si.on_update or []))
                    n_fixed += 1
                new_insts.append(ins)
            blk.instructions[:] = new_insts
    return n_fixed


def _group_geometry(caps):
    """Per-group chunk geometry from per-slot caps."""
    gstart, acc = [], 0
    for gsz in GROUPS:
        gstart.append(acc)
        acc += gsz
    group_of = []
    for g, gsz in enumerate(GROUPS):
        group_of += [g] * gsz
    glo, ghi, lo_off, hi_off, gbase = [], [], [0] * NSLOT, [0] * NSLOT, []
    base = 0
    for g, gsz in enumerate(GROUPS):
        s0 = gstart[g]
        lo_acc = 0
        for s in range(s0, s0 + gsz):
            lo_off[s] = lo_acc
            lo_acc += caps[s][0]
        hi_acc = 0
        for s in range(s0, s0 + gsz):
            hi_off[s] = hi_acc
            hi_acc += caps[s][1]
        glo.append(lo_acc)
        ghi.append(hi_acc)
        gbase.append(base)
        base += lo_acc + hi_acc
    return dict(gstart=gstart, group_of=group_of, glo=glo, ghi=ghi,
                lo_off=lo_off, hi_off=hi_off, gbase=gbase, tot_chunks=base)


def build_nc(caps, hoist=True):
    """Per-core Bass program (SPMD). caps: ((cl, ch) per slot)."""
    caps = list(caps)
    nslot = len(caps)
    geo = _group_geometry(caps)
    gstart, group_of = geo["gstart"], geo["group_of"]
    glo, ghi, gbase = geo["glo"], geo["ghi"], geo["gbase"]
    lo_off, hi_off = geo["lo_off"], geo["hi_off"]
    tot_chunks = geo["tot_chunks"]

    # cst layout: [iota 128 | per-slot dslot cols (tot_chunks) | bias 128 | W 8*128]
    soff = 128
    slot_off = []
    off = 0
    for cl, ch in caps:
        slot_off.append(off)
        off += cl + ch
    boff = soff + tot_chunks
    woff = boff + 128
    cw = woff + 8 * 128

    head_chunks = gbase[HEAD_GROUPS]
    iw_head = head_chunks * 8
    iw_rest = (tot_chunks - head_chunks) * 8

    nc = bacc.Bacc("TRN2", target_bir_lowering=False, debug=False,
                   num_swdge_queues=4)
    x_lo = nc.dram_tensor("x_lo", (LO_MAX, D), f8, kind="ExternalInput")
    x_hi = nc.dram_tensor("x_hi", (N - HI_BASE, D), f8, kind="ExternalInput")
    idx_d = nc.dram_tensor("idx", (128, iw_head + iw_rest), i16,
                           kind="ExternalInput")
    cst_d = nc.dram_tensor("cst", (128, cw), f16, kind="ExternalInput")
    xt_d = nc.dram_tensor("xt", (128, nslot, D), f8, kind="ExternalInput")
    out_d = nc.dram_tensor("out", (128, nslot, D), f16, kind="ExternalOutput")

    with tile.TileContext(nc) as tc:
        with tc.tile_pool(name="const", bufs=1) as cp, \
             tc.tile_pool(name="gather", bufs=2) as gp, \
             tc.tile_pool(name="sel", bufs=4) as sp, \
             tc.tile_pool(name="aggps", bufs=2, space="PSUM") as aps, \
             tc.tile_pool(name="aggT", bufs=3) as atp, \
             tc.tile_pool(name="xtp", bufs=5 if XT_MODE == 2 else 2) as xtp, \
             tc.tile_pool(name="outps", bufs=2, space="PSUM") as ops_, \
             tc.tile_pool(name="outsb", bufs=3 if OSB_MODE == 2 else 2) as osb:

            # Pool-DMA emissions rotate DMASW sem lanes (mod 8) in program
            # order; each lane's sem is locked to one SWDGE queue, so the
            # queue must be emission_index % 4 to stay consistent.
            qn_counter = [0]

            def next_queue():
                q = qn_counter[0] % 4
                qn_counter[0] += 1
                return q

            # SWDGE warmup: tiny gather of row 0 repeated, no input deps
            dummy_idx = cp.tile([128, 8], i16)
            nc.vector.memset(dummy_idx[:], 0)
            dummy_t = cp.tile([128, 1, D], f8)
            nc.gpsimd.dma_gather(
                out_ap=dummy_t[:, 0:1, :], in_ap=x_lo.ap(),
                idxs_ap=dummy_idx[:, 0:8], num_idxs=128, num_idxs_reg=128,
                elem_size=D, queue_num=next_queue(), single_packet=True)

            idx_head = cp.tile([128, iw_head], i16)
            idx_rest = cp.tile([128, iw_rest], i16)
            cst_sb = cp.tile([128, cw], f16)
            nc.sync.dma_start(out=idx_head[:],
                              in_=idx_d.ap()[:, 0:iw_head])
            nc.sync.dma_start(out=cst_sb[:, 0:boff],
                              in_=cst_d.ap()[:, 0:boff])
            nc.sync.dma_start(out=idx_rest[:],
                              in_=idx_d.ap()[:, iw_head:iw_head + iw_rest])
            nc.sync.dma_start(out=cst_sb[:, boff:cw],
                              in_=cst_d.ap()[:, boff:cw])

            iota_b = cst_sb[:, 0:128][:, None, :]

            def emit_group_gather(g):
                gl, gh = glo[g], ghi[g]
                if g < HEAD_GROUPS:
                    isb, o8 = idx_head, gbase[g] * 8
                else:
                    isb, o8 = idx_rest, (gbase[g] - head_chunks) * 8
                t_tile = gp.tile([128, gl + gh, D], f8)
                def spans(n):
                    a = 0
                    while a < n:
                        b = min(a + MAX_GCHUNKS, n)
                        yield a, b
                        a = b

                for a, b in spans(gl):
                    nc.gpsimd.dma_gather(
                        out_ap=t_tile[:, a:b, :], in_ap=x_lo.ap(),
                        idxs_ap=isb[:, o8 + a * 8:o8 + b * 8],
                        num_idxs=(b - a) * 128, num_idxs_reg=(b - a) * 128,
                        elem_size=D, queue_num=next_queue(),
                        single_packet=SINGLE_PACKET)
                for a, b in spans(gh):
                    nc.gpsimd.dma_gather(
                        out_ap=t_tile[:, gl + a:gl + b, :], in_ap=x_hi.ap(),
                        idxs_ap=isb[:, o8 + (gl + a) * 8:o8 + (gl + b) * 8],
                        num_idxs=(b - a) * 128, num_idxs_reg=(b - a) * 128,
                        elem_size=D, queue_num=next_queue(),
                        single_packet=SINGLE_PACKET)
                return t_tile

            st = {}
            gt = {}
            xt_g = {}
            osb_g = {}
            for s in range(nslot + 1):
                # ---- stage A (slot s): gather, S, xt, flipped seg
                if s < nslot:
                    g = group_of[s]
                    if s == gstart[g]:
                        gt[g] = emit_group_gather(g)
                        if XT_MODE == 2:
                            for ss in range(GROUPS[g]):
                                xts = xtp.tile([128, 1, D], f8, name="xtg")
                                nc.sync.dma_start(
                                    out=xts[:],
                                    in_=xt_d.ap()[:, gstart[g] + ss:
                                                  gstart[g] + ss + 1, :])
                                xt_g[gstart[g] + ss] = xts
                        else:
                            xt_g[g] = xtp.tile([128, GROUPS[g], D], f8,
                                               name="xtg")
                            if UNBATCH_XT:
                                for ss in range(GROUPS[g]):
                                    nc.sync.dma_start(
                                        out=xt_g[g][:, ss:ss + 1, :],
                                        in_=xt_d.ap()[:, gstart[g] + ss:
                                                      gstart[g] + ss + 1, :])
                            else:
                                nc.sync.dma_start(
                                    out=xt_g[g][:],
                                    in_=xt_d.ap()[:, gstart[g]:
                                                  gstart[g] + GROUPS[g], :])
                    cl, ch = caps[s]
                    cb = cl + ch
                    gi = s - gstart[g]
                    t_tile = gt[g]
                    base_hi = glo[g]
                    s_tile = sp.tile([128, cb, 128], f16)
                    nc.vector.tensor_tensor(
                        out=s_tile[:],
                        in0=iota_b.to_broadcast([128, cb, 128]),
                        in1=cst_sb[:, soff + slot_off[s]:
                                   soff + slot_off[s] + cb]
                            [:, :, None].to_broadcast([128, cb, 128]),
                        op=mybir.AluOpType.is_equal)
                    agg_ps = aps.tile([128, D], f32, space="PSUM")
                    for fb in range(4):
                        for k in range(cb):
                            col = (lo_off[s] + k if k < cl
                                   else base_hi + hi_off[s] + (k - cl))
                            nc.tensor.matmul(
                                out=agg_ps[:, fb * 128:(fb + 1) * 128],
                                lhsT=t_tile[:, col, fb * 128:(fb + 1) * 128],
                                rhs=s_tile[:, k, :],
                                start=(k == 0), stop=(k == cb - 1))
                    aggt_sb = atp.tile([128, D], f16)
                    nc.scalar.copy(out=aggt_sb[:], in_=agg_ps[:])
                    st[s] = dict(aggt_sb=aggt_sb, g=g, gi=gi)

                # ---- stage B (slot s-1): out matmuls, bias, copy, store
                if 0 <= s - 1 < nslot:
                    s1 = s - 1
                    p = st.pop(s1)
                    g1, gi1 = p["g"], p["gi"]
                    xts = xt_g[s1] if XT_MODE == 2 else xt_g[g1]
                    xgi = 0 if XT_MODE == 2 else gi1
                    if OSB_MODE == 2:
                        ot = osb.tile([128, 1, D], f16, name="osbg")
                        ogi = 0
                    else:
                        if s1 == gstart[g1]:
                            osb_g[g1] = osb.tile([128, GROUPS[g1], D], f16,
                                                 name="osbg")
                        ot = osb_g[g1]
                        ogi = gi1
                    out_ps = ops_.tile([128, D], f32, space="PSUM")
                    for c in range(4):
                        reg = out_ps[:, c * 128:(c + 1) * 128]
                        nc.tensor.matmul(
                            out=reg,
                            lhsT=p["aggt_sb"][:, c * 128:(c + 1) * 128],
                            rhs=cst_sb[:, woff + c * 128:woff + (c + 1) * 128],
                            start=True, stop=False)
                        nc.tensor.matmul(
                            out=reg,
                            lhsT=xts[:, xgi, c * 128:(c + 1) * 128],
                            rhs=cst_sb[:, woff + 512 + c * 128:
                                       woff + 512 + (c + 1) * 128],
                            start=False, stop=True)
                    nc.vector.tensor_tensor(
                        out=ot[:, ogi, 0:128], in0=out_ps[:, 0:128],
                        in1=cst_sb[:, boff:boff + 128],
                        op=mybir.AluOpType.add)
                    nc.scalar.copy(out=ot[:, ogi, 128:D], in_=out_ps[:, 128:D])
                    if OSB_MODE == 2 or UNBATCH_OUT:
                        nc.sync.dma_start(
                            out=out_d.ap()[:, s1:s1 + 1, :],
                            in_=ot[:, ogi:ogi + 1, :])
                    elif s1 == gstart[g1] + GROUPS[g1] - 1:
                        nc.sync.dma_start(
                            out=out_d.ap()[:, gstart[g1]:
                                           gstart[g1] + GROUPS[g1], :],
                            in_=ot[:])

    nc.compile()
    if hoist:
        _hoist_extra_waits(nc)
    return nc


def _wrap_idx(vals, nidx):
    vp = np.zeros(nidx, dtype=np.int16)
    vp[:len(vals)] = vals
    w16 = vp.reshape(nidx // 16, 16).T
    return np.tile(w16, (8, 1))


def pack_inputs(x, edge_index, W_s_rel, W_s_root, b_s_root, W_v_rel, W_v_root):
    nblk = NCORES * NSLOT
    x = np.asarray(x, dtype=np.float32)
    xr8 = np.ascontiguousarray(x.reshape(N, D)).astype(np_f8)
    row = np.asarray(edge_index[0]).astype(np.int64)
    col = np.asarray(edge_index[1]).astype(np.int64)

    blk = row >> 7
    dslot = row & 127

    is_lo = col < HI_BASE
    is_hi = col >= LO_MAX
    is_fx = ~is_lo & ~is_hi
    ml = np.bincount(blk[is_lo], minlength=nblk)
    mh = np.bincount(blk[is_hi], minlength=nblk)
    fl = np.bincount(blk[is_fx], minlength=nblk)
    tot = ml + mh + fl

    cmin = np.zeros(nblk, dtype=np.int64)
    for b in range(nblk):
        best = 99
        for CL in range(14):
            k = min(fl[b], CL * 128 - ml[b])
            if k < 0:
                continue
            best = min(best, CL + (-(-(mh[b] + fl[b] - k) // 128)))
        cmin[b] = best

    order = np.argsort(-(cmin * 4096 + tot), kind="stable")
    assign = np.zeros((NCORES, NSLOT), dtype=np.int64)
    caps = []
    kchoice = np.zeros(nblk, dtype=np.int64)
    core_load = np.zeros(NCORES, dtype=np.int64)
    for s in range(NSLOT):
        members = order[s * 8:(s + 1) * 8]
        best = (99, 0, 0)
        for CL in range(14):
            chs = []
            ok = True
            for b in members:
                k = min(fl[b], CL * 128 - ml[b])
                if k < 0:
                    ok = False
                    break
                chs.append(-(-(mh[b] + fl[b] - k) // 128))
            if ok and CL + max(chs) < best[0]:
                best = (CL + max(chs), CL, max(chs))
        _, CL, CH = best
        caps.append((int(CL), int(CH)))
        for b in members:
            kchoice[b] = min(fl[b], CL * 128 - ml[b])
        msz = tot[members]
        free = list(range(NCORES))
        for b in members[np.argsort(-msz, kind="stable")]:
            c = min(free, key=lambda cc: core_load[cc])
            free.remove(c)
            assign[c, s] = b
            core_load[c] += tot[b]
    caps = tuple(caps)
    geo = _group_geometry(caps)
    gstart, gbase = geo["gstart"], geo["gbase"]
    glo = geo["glo"]
    lo_off, hi_off = geo["lo_off"], geo["hi_off"]
    tot_chunks = geo["tot_chunks"]
    group_of = geo["group_of"]
    soff = 128
    boff = soff + tot_chunks
    woff = boff + 128
    cw = woff + 8 * 128
    slot_off = np.cumsum([0] + [cl + ch for cl, ch in caps])[:NSLOT]

    half = is_hi.astype(np.int64)
    fx_idx = np.nonzero(is_fx)[0]
    fx_blk = blk[fx_idx]
    fo = np.argsort(fx_blk, kind="stable")
    fstarts = np.zeros(nblk + 1, dtype=np.int64)
    np.cumsum(np.bincount(fx_blk, minlength=nblk), out=fstarts[1:])
    ranks = np.empty(len(fx_idx), dtype=np.int64)
    ranks[fo] = np.arange(len(fx_idx)) - fstarts[fx_blk[fo]]
    half[fx_idx] = (ranks >= kchoice[fx_blk]).astype(np.int64)

    bh = blk * 2 + half
    counts = np.bincount(bh, minlength=nblk * 2)
    eorder = np.lexsort((col, bh))
    col_s = col[eorder]
    dslot_s = dslot[eorder]
    starts = np.zeros(nblk * 2 + 1, dtype=np.int64)
    np.cumsum(counts, out=starts[1:])

    rels = [W_s_rel, W_v_rel, W_v_rel, W_v_rel]
    roots = [W_s_root, W_v_root, W_v_root, W_v_root]
    cst_common = np.zeros((128, cw), dtype=np.float16)
    cst_common[:, 0:128] = np.arange(128, dtype=np.float16)[None, :]
    cst_common[:, boff:boff + 128] = \
        np.asarray(b_s_root).astype(np.float16)[None, :]
    for c in range(4):
        cst_common[:, woff + c * 128:woff + (c + 1) * 128] = \
            np.asarray(rels[c]).T.astype(np.float16)
        cst_common[:, woff + 512 + c * 128:woff + 512 + (c + 1) * 128] = \
            np.asarray(roots[c]).T.astype(np.float16)

    x_lo = xr8[:LO_MAX]
    x_hi = xr8[HI_BASE:]
    x4 = x.reshape(N, 4, H)

    in_maps = []
    for c in range(NCORES):
        idx_arr = np.zeros((128, tot_chunks * 8), dtype=np.int16)
        cst = cst_common.copy()
        xt = np.zeros((128, NSLOT, D), dtype=np_f8)
        for s in range(NSLOT):
            b = assign[c, s]
            g = group_of[s]
            cl, ch = caps[s]
            # idx columns: group-lo region then group-hi region
            for hh, cap, coff in (
                    (0, cl, gbase[g] + lo_off[s]),
                    (1, ch, gbase[g] + glo[g] + hi_off[s])):
                g2 = b * 2 + hh
                e0, e1 = starts[g2], starts[g2 + 1]
                ncnt = e1 - e0
                assert ncnt <= cap * 128, (s, b, hh, ncnt, cap)
                vals = col_s[e0:e1] - (HI_BASE if hh else 0)
                idx_arr[:, coff * 8:(coff + cap) * 8] = _wrap_idx(
                    vals.astype(np.int16), cap * 128)
            # cst dslot columns: per-slot [cl | ch] layout
            for hh, cap, coff in ((0, cl, slot_off[s]),
                                  (1, ch, slot_off[s] + cl)):
                g2 = b * 2 + hh
                e0, e1 = starts[g2], starts[g2 + 1]
                ncnt = e1 - e0
                sp_ = np.full(cap * 128, -1.0, dtype=np.float16)
                sp_[:ncnt] = dslot_s[e0:e1].astype(np.float16)
                cst[:, soff + coff:soff + coff + cap] = \
                    sp_.reshape(cap, 128).T
            n0 = b * 128
            n1 = min(N, n0 + 128)
            if n1 > n0:
                xpad = np.zeros((128, 4, H), dtype=np.float32)
                xpad[:n1 - n0] = x4[n0:n1]
                xt[:, s, :] = xpad.transpose(2, 1, 0).reshape(128, D) \
                                  .astype(np_f8)
        in_maps.append({
            "x_lo": x_lo, "x_hi": x_hi, "idx": idx_arr, "cst": cst, "xt": xt,
        })
    meta = dict(caps=caps, assign=assign)
    return in_maps, meta


_NC_CACHE = {}
LAST_RESULTS = None


def run(x, edge_index, W_s_rel, W_s_root, b_s_root, W_v_rel, W_v_root,
        trace=False):
    global LAST_RESULTS
    in_maps, meta = pack_inputs(
        x, edge_index, W_s_rel, W_s_root, b_s_root, W_v_rel, W_v_root)
    key = meta["caps"]
    if key not in _NC_CACHE:
        _NC_CACHE[key] = build_nc(key)
    nc = _NC_CACHE[key]
    res = bass_utils.run_bass_kernel_spmd(
        nc, in_maps, core_ids=list(range(NCORES)), trace=trace)
    LAST_RESULTS = res
    assign = meta["assign"]
    out = np.zeros((N, 4, H), dtype=np.float32)
    for c in range(NCORES):
        oc = np.asarray(res.results[c]["out"], dtype=np.float32)
        for s in range(NSLOT):
            n0 = int(assign[c, s]) * 128
            n1 = min(N, n0 + 128)
            if n1 > n0:
                out[n0:n1] = oc[:n1 - n0, s, :].reshape(-1, 4, H)
    return out


def kernel(x, edge_index, W_s_rel, W_s_root, b_s_root, W_v_rel, W_v_root):
    return run(x, edge_index, W_s_rel, W_s_root, b_s_root, W_v_rel, W_v_root,
               trace=bool(os.environ.get("BASS_TRACE")))


# revision 22
# speedup vs baseline: 1.1442x; 1.1442x over previous
"""Trainium2 Bass kernel for EquivariantGraphConvCheap (gnn_message_passing), v10.

v5 + packed gathers and batched aux DMAs:
  - The extended dma_gather ucode in single_packet mode emits all descriptors
    of an instruction as ONE DMA packet; the HW packet ceiling is 64
    descriptors = 1008 indices (~7.8 chunks).  v5 issued 2 per-slot gathers
    (avg 5.2 chunks); v10 packs gathers to 7 chunks each across slot
    boundaries within a (group, half) region (~80 instructions/core), cutting
    Pool-engine descriptor-gen busy time.
  - Slots grouped (GROUPS); xt loads and out stores are one HWDGE DMA per
    group ([128, nslot, D] HBM layout), shrinking Sync-sequencer load.
  - Edge indices sorted ascending within each (slot, half) segment.
  - Pool-DMA queue_num = emission_index % 4 so the tile framework's DMASW
    sem-lane rotation (mod 8) stays queue-consistent.
Everything else as v5 (e3m4 gather payload, flex lo/hi split, fp16 one-hot S,
flipped segment-sum matmuls, fp16 out, DVE bias add).
"""
import os
import numpy as np
import ml_dtypes

import concourse.bacc as bacc
import concourse.mybir as mybir
import concourse.tile as tile
from concourse import bass_utils

# ---- hardcoded problem geometry ----
N = 50000
E = 500000
H = 128
D = 4 * H
NCORES = 8
NSLOT = 49
LO_MAX = 32768
HI_BASE = 25000
GROUPS = tuple(int(x) for x in
               os.environ.get("KGROUPS", "1,2,3,4,6,6,7,7,7,6").split(","))
assert sum(GROUPS) == NSLOT
# 64-descriptor single-packet ceiling = 1008 idxs; 7 chunks = 896 idxs + sem.
MAX_GCHUNKS = int(os.environ.get("KMAXG", "7"))
HEAD_GROUPS = 2

f16 = mybir.dt.float16
f32 = mybir.dt.float32
f8 = mybir.dt.float8e3
i16 = mybir.dt.int16
np_f8 = ml_dtypes.float8_e3m4


def _hoist_extra_waits(nc, max_waits=1):
    n_fixed = 0
    for fn in nc.m.functions:
        for blk in fn.blocks:
            new_insts = []
            for ins in blk.instructions:
                si = ins.sync_info
                if si is not None and si.on_wait and len(si.on_wait) > max_waits:
                    waits = list(si.on_wait)
                    for j, w in enumerate(waits[:-max_waits]):
                        nop = mybir.InstNoOp(
                            name=f"{ins.name}-waitnop{j}", ins=[], outs=[])
                        nop.engine = ins.engine
                        nop.sync_info = mybir.SyncInfo(on_wait=[w], on_update=[])
                        new_insts.append(nop)
                    ins.sync_info = mybir.SyncInfo(
                        on_wait=waits[-max_waits:],
                        on_update=list(si.on_update or []))
                    n_fixed += 1
                new_insts.append(ins)
            blk.instructions[:] = new_insts
    return n_fixed


def _group_geometry(caps):
    """Per-group chunk geometry from per-slot (cl, ch) caps."""
    gstart, acc = [], 0
    for gsz in GROUPS:
        gstart.append(acc)
        acc += gsz
    group_of = []
    for g, gsz in enumerate(GROUPS):
        group_of += [g] * gsz
    glo, ghi, lo_off, hi_off, gbase = [], [], [0] * NSLOT, [0] * NSLOT, []
    base = 0
    for g, gsz in enumerate(GROUPS):
        s0 = gstart[g]
        lo_acc = 0
        for s in range(s0, s0 + gsz):
            lo_off[s] = lo_acc
            lo_acc += caps[s][0]
        hi_acc = 0
        for s in range(s0, s0 + gsz):
            hi_off[s] = hi_acc
            hi_acc += caps[s][1]
        glo.append(lo_acc)
        ghi.append(hi_acc)
        gbase.append(base)
        base += lo_acc + hi_acc
    return dict(gstart=gstart, group_of=group_of, glo=glo, ghi=ghi,
                lo_off=lo_off, hi_off=hi_off, gbase=gbase, tot_chunks=base)


def build_nc(caps, hoist=True):
    """Per-core Bass program (SPMD). caps: ((cl, ch) per slot)."""
    caps = list(caps)
    nslot = len(caps)
    geo = _group_geometry(caps)
    gstart, group_of = geo["gstart"], geo["group_of"]
    glo, ghi, gbase = geo["glo"], geo["ghi"], geo["gbase"]
    lo_off, hi_off = geo["lo_off"], geo["hi_off"]
    tot_chunks = geo["tot_chunks"]

    # cst layout: [iota 128 | per-slot dslot cols (tot_chunks) | bias 128 | W 8*128]
    soff = 128
    slot_off = []
    off = 0
    for cl, ch in caps:
        slot_off.append(off)
        off += cl + ch
    boff = soff + tot_chunks
    woff = boff + 128
    cw = woff + 8 * 128

    head_chunks = gbase[HEAD_GROUPS]
    iw_head = head_chunks * 8
    iw_rest = (tot_chunks - head_chunks) * 8

    nc = bacc.Bacc("TRN2", target_bir_lowering=False, debug=False,
                   num_swdge_queues=4)
    x_lo = nc.dram_tensor("x_lo", (LO_MAX, D), f8, kind="ExternalInput")
    x_hi = nc.dram_tensor("x_hi", (N - HI_BASE, D), f8, kind="ExternalInput")
    idx_d = nc.dram_tensor("idx", (128, iw_head + iw_rest), i16,
                           kind="ExternalInput")
    cst_d = nc.dram_tensor("cst", (128, cw), f16, kind="ExternalInput")
    xt_d = nc.dram_tensor("xt", (128, nslot, D), f8, kind="ExternalInput")
    out_d = nc.dram_tensor("out", (128, nslot, D), f16, kind="ExternalOutput")

    with tile.TileContext(nc) as tc:
        with tc.tile_pool(name="const", bufs=1) as cp, \
             tc.tile_pool(name="gather", bufs=2) as gp, \
             tc.tile_pool(name="sel", bufs=4) as sp, \
             tc.tile_pool(name="aggps", bufs=2, space="PSUM") as aps, \
             tc.tile_pool(name="aggT", bufs=3) as atp, \
             tc.tile_pool(name="xtp", bufs=2) as xtp, \
             tc.tile_pool(name="outps", bufs=2, space="PSUM") as ops_, \
             tc.tile_pool(name="outsb", bufs=2) as osb:

            # Pool-DMA emissions rotate DMASW sem lanes (mod 8) in program
            # order; each lane's sem is locked to one SWDGE queue, so the
            # queue must be emission_index % 4 to stay consistent.
            qn_counter = [0]

            def next_queue():
                q = qn_counter[0] % 4
                qn_counter[0] += 1
                return q

            # SWDGE warmup: tiny gather of row 0 repeated, no input deps
            dummy_idx = cp.tile([128, 8], i16)
            nc.vector.memset(dummy_idx[:], 0)
            dummy_t = cp.tile([128, 1, D], f8)
            nc.gpsimd.dma_gather(
                out_ap=dummy_t[:, 0:1, :], in_ap=x_lo.ap(),
                idxs_ap=dummy_idx[:, 0:8], num_idxs=128, num_idxs_reg=128,
                elem_size=D, queue_num=next_queue(), single_packet=True)

            idx_head = cp.tile([128, iw_head], i16)
            idx_rest = cp.tile([128, iw_rest], i16)
            cst_sb = cp.tile([128, cw], f16)
            nc.sync.dma_start(out=idx_head[:],
                              in_=idx_d.ap()[:, 0:iw_head])
            nc.sync.dma_start(out=cst_sb[:, 0:boff],
                              in_=cst_d.ap()[:, 0:boff])
            nc.sync.dma_start(out=idx_rest[:],
                              in_=idx_d.ap()[:, iw_head:iw_head + iw_rest])
            nc.sync.dma_start(out=cst_sb[:, boff:cw],
                              in_=cst_d.ap()[:, boff:cw])

            iota_b = cst_sb[:, 0:128][:, None, :]

            def emit_group_gather(g):
                gl, gh = glo[g], ghi[g]
                if g < HEAD_GROUPS:
                    isb, o8 = idx_head, gbase[g] * 8
                else:
                    isb, o8 = idx_rest, (gbase[g] - head_chunks) * 8
                t_tile = gp.tile([128, gl + gh, D], f8)

                def spans(n):
                    a = 0
                    while a < n:
                        b = min(a + MAX_GCHUNKS, n)
                        yield a, b
                        a = b

                for a, b in spans(gl):
                    nc.gpsimd.dma_gather(
                        out_ap=t_tile[:, a:b, :], in_ap=x_lo.ap(),
                        idxs_ap=isb[:, o8 + a * 8:o8 + b * 8],
                        num_idxs=(b - a) * 128, num_idxs_reg=(b - a) * 128,
                        elem_size=D, queue_num=next_queue(),
                        single_packet=True)
                for a, b in spans(gh):
                    nc.gpsimd.dma_gather(
                        out_ap=t_tile[:, gl + a:gl + b, :], in_ap=x_hi.ap(),
                        idxs_ap=isb[:, o8 + (gl + a) * 8:o8 + (gl + b) * 8],
                        num_idxs=(b - a) * 128, num_idxs_reg=(b - a) * 128,
                        elem_size=D, queue_num=next_queue(),
                        single_packet=True)
                return t_tile

            st = {}
            gt = {}
            xt_g = {}
            osb_g = {}
            for s in range(nslot + 1):
                # ---- stage A (slot s): gather, S, xt, flipped seg
                if s < nslot:
                    g = group_of[s]
                    if s == gstart[g]:
                        gt[g] = emit_group_gather(g)
                        xt_g[g] = xtp.tile([128, GROUPS[g], D], f8,
                                           name="xtg")
                        nc.sync.dma_start(
                            out=xt_g[g][:],
                            in_=xt_d.ap()[:, gstart[g]:gstart[g] + GROUPS[g], :])
                    cl, ch = caps[s]
                    cb = cl + ch
                    gi = s - gstart[g]
                    t_tile = gt[g]
                    base_hi = glo[g]
                    s_tile = sp.tile([128, cb, 128], f16)
                    nc.vector.tensor_tensor(
                        out=s_tile[:],
                        in0=iota_b.to_broadcast([128, cb, 128]),
                        in1=cst_sb[:, soff + slot_off[s]:
                                   soff + slot_off[s] + cb]
                            [:, :, None].to_broadcast([128, cb, 128]),
                        op=mybir.AluOpType.is_equal)
                    agg_ps = aps.tile([128, D], f32, space="PSUM")
                    for fb in range(4):
                        for k in range(cb):
                            col = (lo_off[s] + k if k < cl
                                   else base_hi + hi_off[s] + (k - cl))
                            nc.tensor.matmul(
                                out=agg_ps[:, fb * 128:(fb + 1) * 128],
                                lhsT=t_tile[:, col, fb * 128:(fb + 1) * 128],
                                rhs=s_tile[:, k, :],
                                start=(k == 0), stop=(k == cb - 1))
                    aggt_sb = atp.tile([128, D], f16)
                    nc.scalar.copy(out=aggt_sb[:], in_=agg_ps[:])
                    st[s] = dict(aggt_sb=aggt_sb, g=g, gi=gi)

                # ---- stage B (slot s-1): out matmuls, bias, copy, store
                if 0 <= s - 1 < nslot:
                    s1 = s - 1
                    p = st.pop(s1)
                    g1, gi1 = p["g"], p["gi"]
                    if s1 == gstart[g1]:
                        osb_g[g1] = osb.tile([128, GROUPS[g1], D], f16,
                                             name="osbg")
                    ot = osb_g[g1]
                    out_ps = ops_.tile([128, D], f32, space="PSUM")
                    for c in range(4):
                        reg = out_ps[:, c * 128:(c + 1) * 128]
                        nc.tensor.matmul(
                            out=reg,
                            lhsT=p["aggt_sb"][:, c * 128:(c + 1) * 128],
                            rhs=cst_sb[:, woff + c * 128:woff + (c + 1) * 128],
                            start=True, stop=False)
                        nc.tensor.matmul(
                            out=reg,
                            lhsT=xt_g[g1][:, gi1, c * 128:(c + 1) * 128],
                            rhs=cst_sb[:, woff + 512 + c * 128:
                                       woff + 512 + (c + 1) * 128],
                            start=False, stop=True)
                    nc.vector.tensor_tensor(
                        out=ot[:, gi1, 0:128], in0=out_ps[:, 0:128],
                        in1=cst_sb[:, boff:boff + 128],
                        op=mybir.AluOpType.add)
                    nc.scalar.copy(out=ot[:, gi1, 128:D], in_=out_ps[:, 128:D])
                    if s1 == gstart[g1] + GROUPS[g1] - 1:
                        nc.sync.dma_start(
                            out=out_d.ap()[:, gstart[g1]:
                                           gstart[g1] + GROUPS[g1], :],
                            in_=ot[:])

    nc.compile()
    if hoist:
        _hoist_extra_waits(nc)
    return nc


def _wrap_idx(vals, nidx):
    vp = np.zeros(nidx, dtype=np.int16)
    vp[:len(vals)] = vals
    w16 = vp.reshape(nidx // 16, 16).T
    return np.tile(w16, (8, 1))


def pack_inputs(x, edge_index, W_s_rel, W_s_root, b_s_root, W_v_rel, W_v_root):
    nblk = NCORES * NSLOT
    x = np.asarray(x, dtype=np.float32)
    xr8 = np.ascontiguousarray(x.reshape(N, D)).astype(np_f8)
    row = np.asarray(edge_index[0]).astype(np.int64)
    col = np.asarray(edge_index[1]).astype(np.int64)

    blk = row >> 7
    dslot = row & 127

    is_lo = col < HI_BASE
    is_hi = col >= LO_MAX
    is_fx = ~is_lo & ~is_hi
    ml = np.bincount(blk[is_lo], minlength=nblk)
    mh = np.bincount(blk[is_hi], minlength=nblk)
    fl = np.bincount(blk[is_fx], minlength=nblk)
    tot = ml + mh + fl

    cmin = np.zeros(nblk, dtype=np.int64)
    for b in range(nblk):
        best = 99
        for CL in range(14):
            k = min(fl[b], CL * 128 - ml[b])
            if k < 0:
                continue
            best = min(best, CL + (-(-(mh[b] + fl[b] - k) // 128)))
        cmin[b] = best

    order = np.argsort(-(cmin * 4096 + tot), kind="stable")
    assign = np.zeros((NCORES, NSLOT), dtype=np.int64)
    caps = []
    kchoice = np.zeros(nblk, dtype=np.int64)
    core_load = np.zeros(NCORES, dtype=np.int64)
    for s in range(NSLOT):
        members = order[s * 8:(s + 1) * 8]
        best = (99, 0, 0)
        for CL in range(14):
            chs = []
            ok = True
            for b in members:
                k = min(fl[b], CL * 128 - ml[b])
                if k < 0:
                    ok = False
                    break
                chs.append(-(-(mh[b] + fl[b] - k) // 128))
            if ok and CL + max(chs) < best[0]:
                best = (CL + max(chs), CL, max(chs))
        _, CL, CH = best
        caps.append((int(CL), int(CH)))
        for b in members:
            kchoice[b] = min(fl[b], CL * 128 - ml[b])
        msz = tot[members]
        free = list(range(NCORES))
        for b in members[np.argsort(-msz, kind="stable")]:
            c = min(free, key=lambda cc: core_load[cc])
            free.remove(c)
            assign[c, s] = b
            core_load[c] += tot[b]
    caps = tuple(caps)
    geo = _group_geometry(caps)
    gbase = geo["gbase"]
    glo = geo["glo"]
    lo_off, hi_off = geo["lo_off"], geo["hi_off"]
    tot_chunks = geo["tot_chunks"]
    group_of = geo["group_of"]
    soff = 128
    boff = soff + tot_chunks
    woff = boff + 128
    cw = woff + 8 * 128
    slot_off = np.cumsum([0] + [cl + ch for cl, ch in caps])[:NSLOT]

    half = is_hi.astype(np.int64)
    fx_idx = np.nonzero(is_fx)[0]
    fx_blk = blk[fx_idx]
    fo = np.argsort(fx_blk, kind="stable")
    fstarts = np.zeros(nblk + 1, dtype=np.int64)
    np.cumsum(np.bincount(fx_blk, minlength=nblk), out=fstarts[1:])
    ranks = np.empty(len(fx_idx), dtype=np.int64)
    ranks[fo] = np.arange(len(fx_idx)) - fstarts[fx_blk[fo]]
    half[fx_idx] = (ranks >= kchoice[fx_blk]).astype(np.int64)

    bh = blk * 2 + half
    counts = np.bincount(bh, minlength=nblk * 2)
    eorder = np.lexsort((col, bh))
    col_s = col[eorder]
    dslot_s = dslot[eorder]
    starts = np.zeros(nblk * 2 + 1, dtype=np.int64)
    np.cumsum(counts, out=starts[1:])

    rels = [W_s_rel, W_v_rel, W_v_rel, W_v_rel]
    roots = [W_s_root, W_v_root, W_v_root, W_v_root]
    cst_common = np.zeros((128, cw), dtype=np.float16)
    cst_common[:, 0:128] = np.arange(128, dtype=np.float16)[None, :]
    cst_common[:, boff:boff + 128] = \
        np.asarray(b_s_root).astype(np.float16)[None, :]
    for c in range(4):
        cst_common[:, woff + c * 128:woff + (c + 1) * 128] = \
            np.asarray(rels[c]).T.astype(np.float16)
        cst_common[:, woff + 512 + c * 128:woff + 512 + (c + 1) * 128] = \
            np.asarray(roots[c]).T.astype(np.float16)

    x_lo = xr8[:LO_MAX]
    x_hi = xr8[HI_BASE:]
    x4 = x.reshape(N, 4, H)

    in_maps = []
    for c in range(NCORES):
        idx_arr = np.zeros((128, tot_chunks * 8), dtype=np.int16)
        cst = cst_common.copy()
        xt = np.zeros((128, NSLOT, D), dtype=np_f8)
        for s in range(NSLOT):
            b = assign[c, s]
            g = group_of[s]
            cl, ch = caps[s]
            # idx columns: group-lo region then group-hi region
            for hh, cap, coff in (
                    (0, cl, gbase[g] + lo_off[s]),
                    (1, ch, gbase[g] + glo[g] + hi_off[s])):
                g2 = b * 2 + hh
                e0, e1 = starts[g2], starts[g2 + 1]
                ncnt = e1 - e0
                assert ncnt <= cap * 128, (s, b, hh, ncnt, cap)
                vals = col_s[e0:e1] - (HI_BASE if hh else 0)
                idx_arr[:, coff * 8:(coff + cap) * 8] = _wrap_idx(
                    vals.astype(np.int16), cap * 128)
            # cst dslot columns: per-slot [cl | ch] layout
            for hh, cap, coff in ((0, cl, slot_off[s]),
                                  (1, ch, slot_off[s] + cl)):
                g2 = b * 2 + hh
                e0, e1 = starts[g2], starts[g2 + 1]
                ncnt = e1 - e0
                sp_ = np.full(cap * 128, -1.0, dtype=np.float16)
                sp_[:ncnt] = dslot_s[e0:e1].astype(np.float16)
                cst[:, soff + coff:soff + coff + cap] = \
                    sp_.reshape(cap, 128).T
            n0 = b * 128
            n1 = min(N, n0 + 128)
            if n1 > n0:
                xpad = np.zeros((128, 4, H), dtype=np.float32)
                xpad[:n1 - n0] = x4[n0:n1]
                xt[:, s, :] = xpad.transpose(2, 1, 0).reshape(128, D) \
                                  .astype(np_f8)
        in_maps.append({
            "x_lo": x_lo, "x_hi": x_hi, "idx": idx_arr, "cst": cst, "xt": xt,
        })
    meta = dict(caps=caps, assign=assign)
    return in_maps, meta


_NC_CACHE = {}
LAST_RESULTS = None


def run(x, edge_index, W_s_rel, W_s_root, b_s_root, W_v_rel, W_v_root,
        trace=False):
    global LAST_RESULTS
    in_maps, meta = pack_inputs(
        x, edge_index, W_s_rel, W_s_root, b_s_root, W_v_rel, W_v_root)
    key = meta["caps"]
    if key not in _NC_CACHE:
        _NC_CACHE[key] = build_nc(key)
    nc = _NC_CACHE[key]
    res = bass_utils.run_bass_kernel_spmd(
        nc, in_maps, core_ids=list(range(NCORES)), trace=trace)
    LAST_RESULTS = res
    assign = meta["assign"]
    out = np.zeros((N, 4, H), dtype=np.float32)
    for c in range(NCORES):
        oc = np.asarray(res.results[c]["out"], dtype=np.float32)
        for s in range(NSLOT):
            n0 = int(assign[c, s]) * 128
            n1 = min(N, n0 + 128)
            if n1 > n0:
                out[n0:n1] = oc[:n1 - n0, s, :].reshape(-1, 4, H)
    return out


def kernel(x, edge_index, W_s_rel, W_s_root, b_s_root, W_v_rel, W_v_root):
    return run(x, edge_index, W_s_rel, W_s_root, b_s_root, W_v_rel, W_v_root,
               trace=bool(os.environ.get("BASS_TRACE")))
